# revision 24
# baseline (speedup 1.0000x reference)
import time

import numpy as np

import concourse.bacc as bacc
import concourse.mybir as mybir
import concourse.tile as tile
from concourse.bass_utils import run_bass_kernel_spmd

F32 = mybir.dt.float32
F32R = mybir.dt.float32r
BF16 = mybir.dt.bfloat16
AF = mybir.ActivationFunctionType
OP = mybir.AluOpType

FULL = dict(B=2, T=4096, D=2048, H=32, KV=8, DH=64, W=1024, BASE=10000.0)
BIGNEG = -1e30


def _derived(cfg):
    d = dict(cfg)
    d["CH"] = cfg["T"] // 4
    d["KB"] = cfg["W"] // 128
    d["DT"] = cfg["D"] // 128
    d["NP"] = cfg["H"] // 2
    d["NC"] = [(i, min(512, d["CH"] - i)) for i in range(0, d["CH"], 512)]
    assert d["NP"] * 128 == cfg["D"] and d["NP"] % 4 == 0
    return d


def build(cfg):
    c = _derived(cfg)
    CH, KB, DT, NP, KV, H = c["CH"], c["KB"], c["DT"], c["NP"], c["KV"], c["H"]
    NC = c["NC"]
    hpkv = H // KV
    OH = c["D"] // 2
    nc = bacc.Bacc("TRN2", target_bir_lowering=False, debug=False)

    xT = nc.dram_tensor("xT", [128, DT, CH], BF16, kind="ExternalInput")
    wqT = nc.dram_tensor("wqT", [128, NP, DT, 128], BF16, kind="ExternalInput")
    woT = nc.dram_tensor("woT", [128, NP, c["D"]], BF16, kind="ExternalInput")
    ktd = nc.dram_tensor("ktd", [128, KV, KB, 128], BF16, kind="ExternalInput")
    vaug = nc.dram_tensor("vaug", [128, KV, KB, 65], BF16,
                          kind="ExternalInput")
    cosT = nc.dram_tensor("cosT", [128, CH], BF16, kind="ExternalInput")
    sinT = nc.dram_tensor("sinT", [128, CH], BF16, kind="ExternalInput")
    tri = nc.dram_tensor("tri", [128, 128], BF16, kind="ExternalInput")
    brow = nc.dram_tensor("brow", [KB, CH], BF16, kind="ExternalInput")
    sel = nc.dram_tensor("sel", [KB, KB, 128], BF16, kind="ExternalInput")
    sel2 = nc.dram_tensor("sel2", [128, 256], BF16, kind="ExternalInput")
    out = nc.dram_tensor("out", [CH, c["D"]], F32, kind="ExternalOutput")

    swap = [i ^ 1 for i in range(32)]

    with nc.allow_low_precision(reason="bf16 matmuls are intended"), \
         tile.TileContext(nc) as tc:
        with (
            tc.tile_pool(name="consts", bufs=1) as cp,
            tc.tile_pool(name="qt", bufs=4) as qtp,
            tc.tile_pool(name="at", bufs=1) as atp,
            tc.tile_pool(name="wo", bufs=1) as wop,
        ):
            qts = {}
            wqs = {}
            ats = []

            def phase_a(m, rp, xts, cos_sb, sin_sb, psq):
                wq_m = wqs.pop(m)
                qt = qtp.tile([128, CH], BF16, tag="qt")
                qts[m] = qt
                for n0, nn in NC:
                    qp = psq.tile([128, nn], F32, tag="qp", name=f"qp{m}_{n0}")
                    for kt in range(DT):
                        nc.tensor.matmul(
                            qp[:], wq_m[:, kt, :], xts[:, kt, n0:n0 + nn],
                            start=(kt == 0), stop=(kt == DT - 1))
                    qcp = rp.tile([128, nn], BF16, tag="qcp")
                    nc.vector.tensor_copy(qcp[:], qp[:])
                    t1 = rp.tile([128, nn], F32, tag="t1")
                    nc.vector.tensor_mul(t1[:], qp[:], cos_sb[:, n0:n0 + nn])
                    qs = rp.tile([128, nn], BF16, tag="qs")
                    nc.vector.stream_shuffle(qs[:], qcp[:], swap)
                    t2 = rp.tile([128, nn], BF16, tag="t2")
                    nc.vector.tensor_mul(t2[:], qs[:], sin_sb[:, n0:n0 + nn])
                    nc.vector.tensor_add(qt[:, n0:n0 + nn], t1[:], t2[:])

            def phase_b(m, ep, rcp, consts, pss, psa):
                ktd_sb, va_sb, tri_sb, br_sb, sel_sb, sel2_sb = consts
                qt = qts.pop(m)
                kv0 = (2 * m) // hpkv
                kv1 = (2 * m + 1) // hpkv
                at = atp.tile([128, CH], BF16, tag=f"at{m}")
                ats.append(at)
                for n0, nn in NC:
                    avA = psa.tile([128, nn], F32, tag="avA",
                                   name=f"avA{m}_{n0}")
                    avB = psa.tile([128, nn], F32, tag="avB",
                                   name=f"avB{m}_{n0}")
                    pend = []
                    for kb in range(KB):
                        for hh in range(2):
                            kv = kv0 if hh == 0 else kv1
                            lh = ktd_sb[64 * hh:64 * (hh + 1), kv, kb, :]
                            rh = qt[64 * hh:64 * (hh + 1), n0:n0 + nn]
                            sp = pss.tile([128, nn], F32, tag="sp",
                                          name=f"sp{m}_{n0}_{kb}_{hh}")
                            masked = 128 * kb > n0
                            nc.tensor.matmul(sp[:], lh, rh,
                                             start=True, stop=not masked)
                            if masked:
                                mb = min(128 * kb, n0 + nn)
                                nc.tensor.matmul(
                                    sp[:, 0:mb - n0], sel_sb[:, kb, :],
                                    br_sb[:, n0:mb], start=False, stop=True)
                            er = ep.tile([128, nn], BF16, tag="er")
                            nc.scalar.activation(er[:], sp[:], AF.Exp)
                            if n0 <= 128 * kb < n0 + nn:
                                dsl = slice(128 * kb - n0,
                                            128 * kb - n0 + 128)
                                nc.gpsimd.tensor_mul(
                                    er[:, dsl], er[:, dsl], tri_sb[:])

                            def av(hh=hh, kb=kb, kv=kv, er=er):
                                av_t = avA if hh == 0 else avB
                                nc.tensor.matmul(
                                    av_t[0:65, :], va_sb[:, kv, kb, :],
                                    er[:],
                                    start=(kb == 0), stop=(kb == KB - 1))
                            if pend:
                                pend.pop(0)()
                            pend.append(av)
                    for f in pend:
                        f()

                    RA = rcp.tile([128, nn], BF16, tag="recA")
                    nc.vector.reciprocal(RA[0:1, :], avA[64:65, :])
                    RB = rcp.tile([128, nn], BF16, tag="recB")
                    nc.vector.reciprocal(RB[0:1, :], avB[64:65, :])
                    RD = rcp.tile([128, nn], BF16, tag="recD")
                    nc.vector.tensor_sub(RD[0:1, :], RB[0:1, :], RA[0:1, :])
                    bct = pss.tile([128, nn], F32, tag="sp",
                                   name=f"bc{m}_{n0}")
                    nc.tensor.matmul(bct[:], sel2_sb[0:1, 0:128],
                                     RA[0:1, :], start=True, stop=False)
                    nc.tensor.matmul(bct[:], sel2_sb[0:1, 128:256],
                                     RD[0:1, :], start=False, stop=True)
                    bcs = rcp.tile([128, nn], BF16, tag="bcs")
                    nc.vector.tensor_copy(bcs[:], bct[:])
                    nc.vector.tensor_mul(at[0:64, n0:n0 + nn],
                                         avA[0:64, :], bcs[0:64, :])
                    nc.vector.tensor_mul(at[64:128, n0:n0 + nn],
                                         avB[0:64, :], bcs[64:128, :])

            with (
                tc.tile_pool(name="ab", bufs=1) as abp,
                tc.tile_pool(name="wq", bufs=3) as wp,
                tc.tile_pool(name="rope", bufs=3) as rp,
                tc.tile_pool(name="expp", bufs=6) as ep,
                tc.tile_pool(name="rec", bufs=2) as rcp,
                tc.tile_pool(name="psq", bufs=2, space="PSUM") as psq,
                tc.tile_pool(name="pss", bufs=2, space="PSUM") as pss,
                tc.tile_pool(name="psav", bufs=2, space="PSUM") as psa,
            ):
                def load_wq(m):
                    t = wp.tile([128, DT, 128], BF16, tag="wq")
                    nc.sync.dma_start(t[:], wqT[:, m, :, :])
                    wqs[m] = t

                load_wq(0)
                load_wq(1)
                xts = abp.tile([128, DT, CH], BF16)
                for kt in range(DT):
                    nc.sync.dma_start(xts[:, kt, :], xT[:, kt, :])
                cos_sb = abp.tile([128, CH], BF16)
                nc.sync.dma_start(cos_sb[:], cosT[:])
                sin_sb = abp.tile([128, CH], BF16)
                nc.sync.dma_start(sin_sb[:], sinT[:])
                load_wq(2)
                phase_a(0, rp, xts, cos_sb, sin_sb, psq)
                phase_a(1, rp, xts, cos_sb, sin_sb, psq)
                ktd_sb = cp.tile([128, KV, KB, 128], BF16)
                nc.sync.dma_start(ktd_sb[:], ktd[:])
                va_sb = cp.tile([128, KV, KB, 65], BF16)
                nc.sync.dma_start(va_sb[:], vaug[:])
                tri_sb = cp.tile([128, 128], BF16)
                nc.sync.dma_start(tri_sb[:], tri[:])
                br_sb = cp.tile([KB, CH], BF16)
                nc.sync.dma_start(br_sb[:], brow[:])
                sel_sb = cp.tile([KB, KB, 128], BF16)
                nc.sync.dma_start(sel_sb[:], sel[:])
                sel2_sb = cp.tile([128, 256], BF16)
                nc.sync.dma_start(sel2_sb[:], sel2[:])
                wo_h0 = wop.tile([128, NP, OH], BF16, tag="wo0")
                nc.sync.dma_start(wo_h0[:], woT[:, :, 0:OH])
                consts = (ktd_sb, va_sb, tri_sb, br_sb, sel_sb, sel2_sb)
                for m in range(NP):
                    if m + 3 < NP:
                        load_wq(m + 3)
                    if m + 2 < NP:
                        phase_a(m + 2, rp, xts, cos_sb, sin_sb, psq)
                    phase_b(m, ep, rcp, consts, pss, psa)

            with (
                tc.tile_pool(name="osb", bufs=3) as op_,
                tc.tile_pool(name="wo2", bufs=1) as wop2,
                tc.tile_pool(name="psc", bufs=3, space="PSUM") as psc,
            ):
                wo_h1 = wop2.tile([128, NP, OH], BF16, tag="wo1")
                nc.sync.dma_start(wo_h1[:], woT[:, :, OH:2 * OH])
                MQ = CH // 128
                for nh in range(2):
                    wo_h = wo_h0 if nh == 0 else wo_h1
                    for mq in range(MQ):
                        qsl = slice(128 * mq, 128 * (mq + 1))
                        for o0 in range(0, OH, 512):
                            ow = min(512, OH - o0)
                            opx = psc.tile([128, ow], F32, tag="opx")
                            for kq in range(NP):
                                nc.tensor.matmul(
                                    opx[:], ats[kq][:, qsl],
                                    wo_h[:, kq, o0:o0 + ow],
                                    start=(kq == 0), stop=(kq == NP - 1))
                            osb = op_.tile([128, ow], F32, tag="os")
                            nc.vector.tensor_copy(osb[:], opx[:])
                            nc.sync.dma_start(
                                out[qsl, OH * nh + o0:OH * nh + o0 + ow],
                                osb[:])
    nc.compile()
    return nc


def host_inputs(cfg, x, k_cache, v_cache, Wq, Wo, core):
    import ml_dtypes
    bf16 = ml_dtypes.bfloat16
    c = _derived(cfg)
    CH, KB, KV, W, DH, DT, NP = (c["CH"], c["KB"], c["KV"], c["W"], c["DH"],
                                 c["DT"], c["NP"])
    b, ch = core // 4, core % 4
    Tc = k_cache.shape[2]
    f32 = np.float32

    xchunk = x[b, CH * ch:CH * (ch + 1), :].T.astype(f32)
    xT = np.ascontiguousarray(
        xchunk.reshape(DT, 128, CH).transpose(1, 0, 2)).astype(bf16)
    wq_s = (Wq.T.astype(f32) * f32(1.0 / np.sqrt(DH)))
    wqT = np.ascontiguousarray(
        wq_s.reshape(DT, 128, NP, 128).transpose(1, 2, 0, 3)).astype(bf16)
    woT = np.ascontiguousarray(
        Wo.T.astype(f32).reshape(NP, 128, c["D"]).transpose(1, 0, 2)
    ).astype(bf16)
    kw = k_cache[b, :, Tc - W:, :].astype(f32)
    kT64 = kw.reshape(KV, KB, 128, DH).transpose(3, 0, 1, 2)
    ktdm = np.empty((128, KV, KB, 128), f32)
    ktdm[0:64] = kT64
    ktdm[64:128] = kT64
    vw = v_cache[b, :, Tc - W:, :].astype(f32).reshape(KV, KB, 128, DH)
    vp = vw.transpose(2, 0, 1, 3)
    vaugm = np.ones((128, KV, KB, 65), f32)
    vaugm[:, :, :, :DH] = vp
    pos = (CH * ch + np.arange(CH)).astype(f32)
    inv = 1.0 / (cfg["BASE"] ** (np.arange(0, DH, 2, dtype=f32) / DH))
    r = np.arange(128)
    u = (r % 64) // 2
    ang = pos[None, :] * inv[u][:, None]
    cosT = np.cos(ang).astype(bf16)
    sinT = (np.sin(ang) * np.where(r % 2 == 0, -1.0, 1.0)[:, None]
            ).astype(bf16)
    if ch == 0:
        trim = (np.arange(128)[:, None] <= np.arange(128)[None, :]
                ).astype(f32)
        browm = np.zeros((KB, CH), f32)
        for kb in range(KB):
            browm[kb, :128 * kb] = BIGNEG
    else:
        trim = np.ones((128, 128), f32)
        browm = np.zeros((KB, CH), f32)
    selm = np.zeros((KB, KB, 128), f32)
    for kb in range(KB):
        selm[kb, kb, :] = 1.0
    sel2 = np.zeros((128, 256), f32)
    sel2[0, 0:128] = 1.0
    sel2[0, 192:256] = 1.0
    sel2 = sel2.astype(bf16)
    return {"xT": xT, "wqT": wqT, "woT": woT, "ktd": ktdm.astype(bf16),
            "vaug": vaugm.astype(bf16),
            "cosT": cosT, "sinT": sinT,
            "tri": trim.astype(bf16), "brow": browm.astype(bf16),
            "sel": selm.astype(bf16), "sel2": sel2}


_NC_CACHE = {}


def run(cfg, x, k_cache, v_cache, Wq, Wo, trace=False):
    key = tuple(sorted((k, v) for k, v in cfg.items()))
    if key not in _NC_CACHE:
        _NC_CACHE[key] = build(cfg)
    nc = _NC_CACHE[key]
    in_maps = [host_inputs(cfg, x, k_cache, v_cache, Wq, Wo, c)
               for c in range(8)]
    res = None
    for attempt in range(3):
        try:
            res = run_bass_kernel_spmd(nc, in_maps, core_ids=list(range(8)),
                                       trace=trace)
            break
        except Exception:
            if attempt == 2:
                raise
            time.sleep(2.0)
    outs = [res.results[c]["out"] for c in range(8)]
    full = np.stack([np.concatenate(outs[0:4], axis=0),
                     np.concatenate(outs[4:8], axis=0)])
    return full, res


def kernel(x, k_cache, v_cache, Wq, Wo):
    full, _ = run(FULL, np.asarray(x), np.asarray(k_cache),
                  np.asarray(v_cache), np.asarray(Wq), np.asarray(Wo))
    return full.astype(np.float32)


# revision 26
# speedup vs baseline: 1.2675x; 1.2675x over previous
import time

import numpy as np

import concourse.bacc as bacc
import concourse.mybir as mybir
import concourse.tile as tile
from concourse.bass_utils import run_bass_kernel_spmd

F32 = mybir.dt.float32
F32R = mybir.dt.float32r
BF16 = mybir.dt.bfloat16
AF = mybir.ActivationFunctionType
OP = mybir.AluOpType

FULL = dict(B=2, T=4096, D=2048, H=32, KV=8, DH=64, W=1024, BASE=10000.0)
BIGNEG = -1e30


def _derived(cfg):
    d = dict(cfg)
    d["CH"] = cfg["T"] // 4
    d["KB"] = cfg["W"] // 128
    d["DT"] = cfg["D"] // 128
    d["NP"] = cfg["H"] // 2
    d["NC"] = [(i, min(512, d["CH"] - i)) for i in range(0, d["CH"], 512)]
    assert d["NP"] * 128 == cfg["D"] and d["NP"] % 4 == 0
    return d


def build(cfg):
    c = _derived(cfg)
    CH, KB, DT, NP, KV, H = c["CH"], c["KB"], c["DT"], c["NP"], c["KV"], c["H"]
    NC = c["NC"]
    hpkv = H // KV
    OH = c["D"] // 2
    AUG = 64 + KB
    nc = bacc.Bacc("TRN2", target_bir_lowering=False, debug=False)

    xT = nc.dram_tensor("xT", [128, DT, CH], BF16, kind="ExternalInput")
    wqT = nc.dram_tensor("wqT", [128, NP, DT, 128], BF16, kind="ExternalInput")
    woT = nc.dram_tensor("woT", [128, NP, c["D"]], BF16, kind="ExternalInput")
    kaug = nc.dram_tensor("kaug", [AUG, KV, KB, 128], BF16,
                          kind="ExternalInput")
    vaug = nc.dram_tensor("vaug", [128, KV, KB, 65], BF16,
                          kind="ExternalInput")
    cosT = nc.dram_tensor("cosT", [128, CH], BF16, kind="ExternalInput")
    sinT = nc.dram_tensor("sinT", [128, CH], BF16, kind="ExternalInput")
    tri = nc.dram_tensor("tri", [128, 128], BF16, kind="ExternalInput")
    brow = nc.dram_tensor("brow", [KB, CH], BF16, kind="ExternalInput")
    sel2 = nc.dram_tensor("sel2", [128, 256], BF16, kind="ExternalInput")
    out = nc.dram_tensor("out", [CH, c["D"]], F32, kind="ExternalOutput")

    swap = [i ^ 1 for i in range(32)]

    with nc.allow_low_precision(reason="bf16 matmuls are intended"), \
         tile.TileContext(nc) as tc:
        with (
            tc.tile_pool(name="consts", bufs=1) as cp,
            tc.tile_pool(name="qa", bufs=8) as qap,
            tc.tile_pool(name="at", bufs=1) as atp,
            tc.tile_pool(name="wo", bufs=1) as wop,
        ):
            qas = {}
            wqs = {}
            ats = []
            den_pending = []

            def phase_a(m, rp, xts, cos_sb, sin_sb, br_sb, psq):
                wq_m = wqs.pop(m)
                qaA = qap.tile([AUG, CH], BF16, tag="qa")
                qaB = qap.tile([AUG, CH], BF16, tag="qa")
                qas[m] = (qaA, qaB)
                nc.sync.dma_start(qaA[64:AUG, :], br_sb[:])
                nc.sync.dma_start(qaB[64:AUG, :], br_sb[:])
                for n0, nn in NC:
                    qp = psq.tile([128, nn], F32, tag="qp", name=f"qp{m}_{n0}")
                    for kt in range(DT):
                        nc.tensor.matmul(
                            qp[:], wq_m[:, kt, :], xts[:, kt, n0:n0 + nn],
                            start=(kt == 0), stop=(kt == DT - 1))
                    qcp = rp.tile([128, nn], BF16, tag="qcp")
                    nc.vector.tensor_copy(qcp[:], qp[:])
                    t1 = rp.tile([128, nn], F32, tag="t1")
                    nc.vector.tensor_mul(t1[:], qp[:], cos_sb[:, n0:n0 + nn])
                    qs = rp.tile([128, nn], BF16, tag="qs")
                    nc.vector.stream_shuffle(qs[:], qcp[:], swap)
                    t2 = rp.tile([128, nn], BF16, tag="t2")
                    nc.vector.tensor_mul(t2[:], qs[:], sin_sb[:, n0:n0 + nn])
                    nc.vector.tensor_add(qaA[0:64, n0:n0 + nn],
                                         t1[0:64, :], t2[0:64, :])
                    nc.vector.tensor_add(qaB[0:64, n0:n0 + nn],
                                         t1[64:128, :], t2[64:128, :])

            def phase_b(m, ep, rcp, consts, pss, psa):
                ka_sb, va_sb, tri_sb, sel2_sb = consts
                qaA, qaB = qas.pop(m)
                kv0 = (2 * m) // hpkv
                kv1 = (2 * m + 1) // hpkv
                at = atp.tile([128, CH], BF16, tag=f"at{m}")
                ats.append(at)
                for n0, nn in NC:
                    avA = psa.tile([128, nn], F32, tag="avA",
                                   name=f"avA{m}_{n0}")
                    avB = psa.tile([128, nn], F32, tag="avB",
                                   name=f"avB{m}_{n0}")
                    pend = []
                    slot = 0
                    for kb in range(KB):
                        for hh in range(2):
                            kv = kv0 if hh == 0 else kv1
                            qa = qaA if hh == 0 else qaB
                            sp = pss.tile([128, nn], F32, tag="sp",
                                          name=f"sp{m}_{n0}_{kb}_{hh}")
                            nc.tensor.matmul(sp[:], ka_sb[:, kv, kb, :],
                                             qa[:, n0:n0 + nn],
                                             start=True, stop=True)
                            er = ep.tile([128, nn], BF16, tag="er")
                            nc.scalar.activation(er[:], sp[:], AF.Exp)
                            if n0 <= 128 * kb < n0 + nn:
                                dsl = slice(128 * kb - n0,
                                            128 * kb - n0 + 128)
                                nc.gpsimd.tensor_mul(
                                    er[:, dsl], er[:, dsl], tri_sb[:])

                            def av(hh=hh, kb=kb, kv=kv, er=er):
                                av_t = avA if hh == 0 else avB
                                nc.tensor.matmul(
                                    av_t[0:65, :], va_sb[:, kv, kb, :],
                                    er[:],
                                    start=(kb == 0), stop=(kb == KB - 1))
                            if pend:
                                pend.pop(0)()
                            pend.append(av)
                            slot += 1
                            if slot == 4:
                                while den_pending:
                                    den_pending.pop(0)()
                    for f in pend:
                        f()

                    def den(avA=avA, avB=avB, n0=n0, nn=nn, at=at):
                        RA = rcp.tile([128, nn], BF16, tag="recA")
                        nc.vector.reciprocal(RA[0:1, :], avA[64:65, :])
                        RB = rcp.tile([128, nn], BF16, tag="recB")
                        nc.vector.reciprocal(RB[0:1, :], avB[64:65, :])
                        RD = rcp.tile([128, nn], BF16, tag="recD")
                        nc.vector.tensor_sub(RD[0:1, :], RB[0:1, :],
                                             RA[0:1, :])
                        bct = pss.tile([128, nn], F32, tag="sp",
                                       name=f"bc{m}_{n0}")
                        nc.tensor.matmul(bct[:], sel2_sb[0:1, 0:128],
                                         RA[0:1, :], start=True, stop=False)
                        nc.tensor.matmul(bct[:], sel2_sb[0:1, 128:256],
                                         RD[0:1, :], start=False, stop=True)
                        bcs = rcp.tile([128, nn], BF16, tag="bcs")
                        nc.vector.tensor_copy(bcs[:], bct[:])
                        nc.vector.tensor_mul(at[0:64, n0:n0 + nn],
                                             avA[0:64, :], bcs[0:64, :])
                        nc.vector.tensor_mul(at[64:128, n0:n0 + nn],
                                             avB[0:64, :], bcs[64:128, :])
                    den_pending.append(den)

            with (
                tc.tile_pool(name="ab", bufs=1) as abp,
                tc.tile_pool(name="wq", bufs=3) as wp,
                tc.tile_pool(name="rope", bufs=3) as rp,
                tc.tile_pool(name="expp", bufs=6) as ep,
                tc.tile_pool(name="rec", bufs=2) as rcp,
                tc.tile_pool(name="psq", bufs=2, space="PSUM") as psq,
                tc.tile_pool(name="pss", bufs=2, space="PSUM") as pss,
                tc.tile_pool(name="psav", bufs=2, space="PSUM") as psa,
            ):
                def load_wq(m):
                    t = wp.tile([128, DT, 128], BF16, tag="wq")
                    nc.sync.dma_start(t[:], wqT[:, m, :, :])
                    wqs[m] = t

                load_wq(0)
                load_wq(1)
                xts = abp.tile([128, DT, CH], BF16)
                for kt in range(DT):
                    nc.sync.dma_start(xts[:, kt, :], xT[:, kt, :])
                cos_sb = abp.tile([128, CH], BF16)
                nc.sync.dma_start(cos_sb[:], cosT[:])
                sin_sb = abp.tile([128, CH], BF16)
                nc.sync.dma_start(sin_sb[:], sinT[:])
                br_sb = cp.tile([KB, CH], BF16)
                nc.sync.dma_start(br_sb[:], brow[:])
                load_wq(2)
                phase_a(0, rp, xts, cos_sb, sin_sb, br_sb, psq)
                phase_a(1, rp, xts, cos_sb, sin_sb, br_sb, psq)
                ka_sb = cp.tile([AUG, KV, KB, 128], BF16)
                nc.sync.dma_start(ka_sb[:], kaug[:])
                va_sb = cp.tile([128, KV, KB, 65], BF16)
                nc.sync.dma_start(va_sb[:], vaug[:])
                tri_sb = cp.tile([128, 128], BF16)
                nc.sync.dma_start(tri_sb[:], tri[:])
                sel2_sb = cp.tile([128, 256], BF16)
                nc.sync.dma_start(sel2_sb[:], sel2[:])
                wo_h0 = wop.tile([128, NP, OH], BF16, tag="wo0")
                nc.sync.dma_start(wo_h0[:], woT[:, :, 0:OH])
                consts = (ka_sb, va_sb, tri_sb, sel2_sb)
                for m in range(NP):
                    if m + 3 < NP:
                        load_wq(m + 3)
                    if m + 2 < NP:
                        phase_a(m + 2, rp, xts, cos_sb, sin_sb, br_sb, psq)
                    phase_b(m, ep, rcp, consts, pss, psa)
                while den_pending:
                    den_pending.pop(0)()

            with (
                tc.tile_pool(name="osb", bufs=3) as op_,
                tc.tile_pool(name="wo2", bufs=1) as wop2,
                tc.tile_pool(name="psc", bufs=3, space="PSUM") as psc,
            ):
                wo_h1 = wop2.tile([128, NP, OH], BF16, tag="wo1")
                nc.sync.dma_start(wo_h1[:], woT[:, :, OH:2 * OH])
                MQ = CH // 128
                for nh in range(2):
                    wo_h = wo_h0 if nh == 0 else wo_h1
                    for mq in range(MQ):
                        qsl = slice(128 * mq, 128 * (mq + 1))
                        for o0 in range(0, OH, 512):
                            ow = min(512, OH - o0)
                            opx = psc.tile([128, ow], F32, tag="opx")
                            for kq in range(NP):
                                nc.tensor.matmul(
                                    opx[:], ats[kq][:, qsl],
                                    wo_h[:, kq, o0:o0 + ow],
                                    start=(kq == 0), stop=(kq == NP - 1))
                            osb = op_.tile([128, ow], F32, tag="os")
                            nc.vector.tensor_copy(osb[:], opx[:])
                            nc.sync.dma_start(
                                out[qsl, OH * nh + o0:OH * nh + o0 + ow],
                                osb[:])
    nc.compile()
    return nc


def host_inputs(cfg, x, k_cache, v_cache, Wq, Wo, core):
    import ml_dtypes
    bf16 = ml_dtypes.bfloat16
    c = _derived(cfg)
    CH, KB, KV, W, DH, DT, NP = (c["CH"], c["KB"], c["KV"], c["W"], c["DH"],
                                 c["DT"], c["NP"])
    b, ch = core // 4, core % 4
    Tc = k_cache.shape[2]
    f32 = np.float32

    xchunk = x[b, CH * ch:CH * (ch + 1), :].T.astype(f32)
    xT = np.ascontiguousarray(
        xchunk.reshape(DT, 128, CH).transpose(1, 0, 2)).astype(bf16)
    wq_s = (Wq.T.astype(f32) * f32(1.0 / np.sqrt(DH)))
    wqT = np.ascontiguousarray(
        wq_s.reshape(DT, 128, NP, 128).transpose(1, 2, 0, 3)).astype(bf16)
    woT = np.ascontiguousarray(
        Wo.T.astype(f32).reshape(NP, 128, c["D"]).transpose(1, 0, 2)
    ).astype(bf16)
    kw = k_cache[b, :, Tc - W:, :].astype(f32)
    kT64 = kw.reshape(KV, KB, 128, DH).transpose(3, 0, 1, 2)
    kaugm = np.zeros((64 + KB, KV, KB, 128), f32)
    kaugm[0:64] = kT64
    for kb in range(KB):
        kaugm[64 + kb, :, kb, :] = 1.0
    vw = v_cache[b, :, Tc - W:, :].astype(f32).reshape(KV, KB, 128, DH)
    vp = vw.transpose(2, 0, 1, 3)
    vaugm = np.ones((128, KV, KB, 65), f32)
    vaugm[:, :, :, :DH] = vp
    pos = (CH * ch + np.arange(CH)).astype(f32)
    inv = 1.0 / (cfg["BASE"] ** (np.arange(0, DH, 2, dtype=f32) / DH))
    r = np.arange(128)
    u = (r % 64) // 2
    ang = pos[None, :] * inv[u][:, None]
    cosT = np.cos(ang).astype(bf16)
    sinT = (np.sin(ang) * np.where(r % 2 == 0, -1.0, 1.0)[:, None]
            ).astype(bf16)
    if ch == 0:
        trim = (np.arange(128)[:, None] <= np.arange(128)[None, :]
                ).astype(f32)
        browm = np.zeros((KB, CH), f32)
        for kb in range(KB):
            browm[kb, :128 * kb] = BIGNEG
    else:
        trim = np.ones((128, 128), f32)
        browm = np.zeros((KB, CH), f32)
    sel2 = np.zeros((128, 256), f32)
    sel2[0, 0:128] = 1.0
    sel2[0, 192:256] = 1.0
    return {"xT": xT, "wqT": wqT, "woT": woT,
            "kaug": kaugm.astype(bf16), "vaug": vaugm.astype(bf16),
            "cosT": cosT, "sinT": sinT,
            "tri": trim.astype(bf16), "brow": browm.astype(bf16),
            "sel2": sel2.astype(bf16)}


_NC_CACHE = {}


def run(cfg, x, k_cache, v_cache, Wq, Wo, trace=False):
    key = tuple(sorted((k, v) for k, v in cfg.items()))
    if key not in _NC_CACHE:
        _NC_CACHE[key] = build(cfg)
    nc = _NC_CACHE[key]
    in_maps = [host_inputs(cfg, x, k_cache, v_cache, Wq, Wo, c)
               for c in range(8)]
    res = None
    for attempt in range(3):
        try:
            res = run_bass_kernel_spmd(nc, in_maps, core_ids=list(range(8)),
                                       trace=trace)
            break
        except Exception:
            if attempt == 2:
                raise
            time.sleep(2.0)
    outs = [res.results[c]["out"] for c in range(8)]
    full = np.stack([np.concatenate(outs[0:4], axis=0),
                     np.concatenate(outs[4:8], axis=0)])
    return full, res


def kernel(x, k_cache, v_cache, Wq, Wo):
    full, _ = run(FULL, np.asarray(x), np.asarray(k_cache),
                  np.asarray(v_cache), np.asarray(Wq), np.asarray(Wo))
    return full.astype(np.float32)


# revision 27
# speedup vs baseline: 1.5128x; 1.1935x over previous
import time

import numpy as np

import concourse.bacc as bacc
import concourse.mybir as mybir
import concourse.tile as tile
from concourse.bass_utils import run_bass_kernel_spmd

F32 = mybir.dt.float32
F32R = mybir.dt.float32r
BF16 = mybir.dt.bfloat16
AF = mybir.ActivationFunctionType
OP = mybir.AluOpType

FULL = dict(B=2, T=4096, D=2048, H=32, KV=8, DH=64, W=1024, BASE=10000.0)
BIGNEG = -1e30


def _derived(cfg):
    d = dict(cfg)
    d["CH"] = cfg["T"] // 4
    d["KB"] = cfg["W"] // 128
    d["DT"] = cfg["D"] // 128
    d["NP"] = cfg["H"] // 2
    d["NC"] = [(i, min(512, d["CH"] - i)) for i in range(0, d["CH"], 512)]
    assert d["NP"] * 128 == cfg["D"] and d["NP"] % 4 == 0
    return d


def build(cfg):
    c = _derived(cfg)
    CH, KB, DT, NP, KV, H = c["CH"], c["KB"], c["DT"], c["NP"], c["KV"], c["H"]
    NC = c["NC"]
    hpkv = H // KV
    OH = c["D"] // 2
    AUG = 64 + KB
    nc = bacc.Bacc("TRN2", target_bir_lowering=False, debug=False)

    xT = nc.dram_tensor("xT", [128, DT, CH], BF16, kind="ExternalInput")
    wqT = nc.dram_tensor("wqT", [128, NP, DT, 128], BF16, kind="ExternalInput")
    woT = nc.dram_tensor("woT", [128, NP, c["D"]], BF16, kind="ExternalInput")
    kaug = nc.dram_tensor("kaug", [AUG, KV, KB, 128], BF16,
                          kind="ExternalInput")
    vaug = nc.dram_tensor("vaug", [128, KV, KB, 65], BF16,
                          kind="ExternalInput")
    cosT = nc.dram_tensor("cosT", [128, CH], BF16, kind="ExternalInput")
    sinT = nc.dram_tensor("sinT", [128, CH], BF16, kind="ExternalInput")
    tri = nc.dram_tensor("tri", [128, 128], BF16, kind="ExternalInput")
    brow = nc.dram_tensor("brow", [KB, CH], BF16, kind="ExternalInput")
    sel2 = nc.dram_tensor("sel2", [128, 256], BF16, kind="ExternalInput")
    out = nc.dram_tensor("out", [CH, c["D"]], F32, kind="ExternalOutput")

    swap = [i ^ 1 for i in range(32)]

    with nc.allow_low_precision(reason="bf16 matmuls are intended"), \
         tile.TileContext(nc) as tc:
        with (
            tc.tile_pool(name="consts", bufs=1) as cp,
            tc.tile_pool(name="qa", bufs=8) as qap,
            tc.tile_pool(name="at", bufs=1) as atp,
            tc.tile_pool(name="wo", bufs=1) as wop,
        ):
            qas = {}
            wqs = {}
            ats = []
            den_pending = []

            def phase_a(m, rp, xts, cos_sb, sin_sb, br_sb, psq):
                wq_m = wqs.pop(m)
                qaA = qap.tile([AUG, CH], BF16, tag="qa")
                qaB = qap.tile([AUG, CH], BF16, tag="qa")
                qas[m] = (qaA, qaB)
                nc.sync.dma_start(qaA[64:AUG, :], br_sb[:])
                nc.sync.dma_start(qaB[64:AUG, :], br_sb[:])
                for n0, nn in NC:
                    qp = psq.tile([128, nn], F32, tag="qp", name=f"qp{m}_{n0}")
                    for kt in range(DT):
                        nc.tensor.matmul(
                            qp[:], wq_m[:, kt, :], xts[:, kt, n0:n0 + nn],
                            start=(kt == 0), stop=(kt == DT - 1))
                    qcp = rp.tile([128, nn], BF16, tag="qcp")
                    nc.vector.tensor_copy(qcp[:], qp[:])
                    t1 = rp.tile([128, nn], F32, tag="t1")
                    nc.vector.tensor_mul(t1[:], qp[:], cos_sb[:, n0:n0 + nn])
                    qs = rp.tile([128, nn], BF16, tag="qs")
                    nc.vector.stream_shuffle(qs[:], qcp[:], swap)
                    t2 = rp.tile([128, nn], BF16, tag="t2")
                    nc.vector.tensor_mul(t2[:], qs[:], sin_sb[:, n0:n0 + nn])
                    nc.vector.tensor_add(qaA[0:64, n0:n0 + nn],
                                         t1[0:64, :], t2[0:64, :])
                    nc.vector.tensor_add(qaB[0:64, n0:n0 + nn],
                                         t1[64:128, :], t2[64:128, :])

            def phase_b(m, ep, rcp, consts, pss, psa):
                ka_sb, va_sb, tri_sb, sel2_sb = consts
                qaA, qaB = qas.pop(m)
                kv0 = (2 * m) // hpkv
                kv1 = (2 * m + 1) // hpkv
                at = atp.tile([128, CH], BF16, tag=f"at{m}")
                ats.append(at)
                for n0, nn in NC:
                    avA = psa.tile([128, nn], F32, tag="avA",
                                   name=f"avA{m}_{n0}")
                    avB = psa.tile([128, nn], F32, tag="avB",
                                   name=f"avB{m}_{n0}")
                    pend = []
                    for kb in range(KB):
                        sp = pss.tile([128, 2 * nn], F32, tag="sp",
                                      name=f"sp{m}_{n0}_{kb}")
                        nc.tensor.matmul(sp[:, 0:nn], ka_sb[:, kv0, kb, :],
                                         qaA[:, n0:n0 + nn],
                                         start=True, stop=True)
                        nc.tensor.matmul(sp[:, nn:2 * nn],
                                         ka_sb[:, kv1, kb, :],
                                         qaB[:, n0:n0 + nn],
                                         start=True, stop=True)
                        er = ep.tile([128, 2 * nn], BF16, tag="er")
                        nc.scalar.activation(er[:], sp[:], AF.Exp)
                        if n0 <= 128 * kb < n0 + nn:
                            d0 = 128 * kb - n0
                            nc.gpsimd.tensor_mul(
                                er[:, d0:d0 + 128], er[:, d0:d0 + 128],
                                tri_sb[:])
                            nc.gpsimd.tensor_mul(
                                er[:, nn + d0:nn + d0 + 128],
                                er[:, nn + d0:nn + d0 + 128], tri_sb[:])

                        def av(kb=kb, er=er):
                            nc.tensor.matmul(
                                avA[0:65, :], va_sb[:, kv0, kb, :],
                                er[:, 0:nn],
                                start=(kb == 0), stop=(kb == KB - 1))
                            nc.tensor.matmul(
                                avB[0:65, :], va_sb[:, kv1, kb, :],
                                er[:, nn:2 * nn],
                                start=(kb == 0), stop=(kb == KB - 1))
                        if kb == 2:
                            while den_pending:
                                den_pending.pop(0)()
                        if len(pend) >= 2:
                            pend.pop(0)()
                        pend.append(av)
                    for f in pend:
                        f()

                    RA = rcp.tile([128, nn], BF16, tag="recA")
                    nc.vector.reciprocal(RA[0:1, :], avA[64:65, :])
                    RB = rcp.tile([128, nn], BF16, tag="recB")
                    nc.vector.reciprocal(RB[0:1, :], avB[64:65, :])
                    RD = rcp.tile([128, nn], BF16, tag="recD")
                    nc.vector.tensor_sub(RD[0:1, :], RB[0:1, :], RA[0:1, :])

                    def den(avA=avA, avB=avB, n0=n0, nn=nn, at=at,
                            RA=RA, RD=RD):
                        bct = pss.tile([128, 2 * nn], F32, tag="sp",
                                       name=f"bc{m}_{n0}")
                        nc.tensor.matmul(bct[:, 0:nn], sel2_sb[0:1, 0:128],
                                         RA[0:1, :], start=True, stop=False)
                        nc.tensor.matmul(bct[:, 0:nn],
                                         sel2_sb[0:1, 128:256],
                                         RD[0:1, :], start=False, stop=True)
                        bcs = rcp.tile([128, nn], BF16, tag="bcs")
                        nc.vector.tensor_copy(bcs[:], bct[:, 0:nn])
                        nc.vector.tensor_mul(at[0:64, n0:n0 + nn],
                                             avA[0:64, :], bcs[0:64, :])
                        nc.vector.tensor_mul(at[64:128, n0:n0 + nn],
                                             avB[0:64, :], bcs[64:128, :])
                    den_pending.append(den)

            with (
                tc.tile_pool(name="ab", bufs=1) as abp,
                tc.tile_pool(name="wq", bufs=3) as wp,
                tc.tile_pool(name="rope", bufs=3) as rp,
                tc.tile_pool(name="expp", bufs=6) as ep,
                tc.tile_pool(name="rec", bufs=2) as rcp,
                tc.tile_pool(name="psq", bufs=2, space="PSUM") as psq,
                tc.tile_pool(name="pss", bufs=2, space="PSUM") as pss,
                tc.tile_pool(name="psav", bufs=1, space="PSUM") as psa,
            ):
                def load_wq(m):
                    t = wp.tile([128, DT, 128], BF16, tag="wq")
                    nc.sync.dma_start(t[:], wqT[:, m, :, :])
                    wqs[m] = t

                load_wq(0)
                load_wq(1)
                xts = abp.tile([128, DT, CH], BF16)
                for kt in range(DT):
                    nc.sync.dma_start(xts[:, kt, :], xT[:, kt, :])
                cos_sb = abp.tile([128, CH], BF16)
                nc.sync.dma_start(cos_sb[:], cosT[:])
                sin_sb = abp.tile([128, CH], BF16)
                nc.sync.dma_start(sin_sb[:], sinT[:])
                br_sb = cp.tile([KB, CH], BF16)
                nc.sync.dma_start(br_sb[:], brow[:])
                load_wq(2)
                phase_a(0, rp, xts, cos_sb, sin_sb, br_sb, psq)
                phase_a(1, rp, xts, cos_sb, sin_sb, br_sb, psq)
                ka_sb = cp.tile([AUG, KV, KB, 128], BF16)
                nc.sync.dma_start(ka_sb[:], kaug[:])
                va_sb = cp.tile([128, KV, KB, 65], BF16)
                nc.sync.dma_start(va_sb[:], vaug[:])
                tri_sb = cp.tile([128, 128], BF16)
                nc.sync.dma_start(tri_sb[:], tri[:])
                sel2_sb = cp.tile([128, 256], BF16)
                nc.sync.dma_start(sel2_sb[:], sel2[:])
                wo_h0 = wop.tile([128, NP, OH], BF16, tag="wo0")
                nc.sync.dma_start(wo_h0[:], woT[:, :, 0:OH])
                consts = (ka_sb, va_sb, tri_sb, sel2_sb)
                for m in range(NP):
                    if m + 3 < NP:
                        load_wq(m + 3)
                    if m + 2 < NP:
                        phase_a(m + 2, rp, xts, cos_sb, sin_sb, br_sb, psq)
                        while den_pending:
                            den_pending.pop(0)()
                    phase_b(m, ep, rcp, consts, pss, psa)
                while den_pending:
                    den_pending.pop(0)()

            with (
                tc.tile_pool(name="osb", bufs=3) as op_,
                tc.tile_pool(name="wo2", bufs=1) as wop2,
                tc.tile_pool(name="psc", bufs=3, space="PSUM") as psc,
            ):
                wo_h1 = wop2.tile([128, NP, OH], BF16, tag="wo1")
                nc.sync.dma_start(wo_h1[:], woT[:, :, OH:2 * OH])
                MQ = CH // 128
                for nh in range(2):
                    wo_h = wo_h0 if nh == 0 else wo_h1
                    for mq in range(MQ):
                        qsl = slice(128 * mq, 128 * (mq + 1))
                        for o0 in range(0, OH, 512):
                            ow = min(512, OH - o0)
                            opx = psc.tile([128, ow], F32, tag="opx")
                            for kq in range(NP):
                                nc.tensor.matmul(
                                    opx[:], ats[kq][:, qsl],
                                    wo_h[:, kq, o0:o0 + ow],
                                    start=(kq == 0), stop=(kq == NP - 1))
                            osb = op_.tile([128, ow], F32, tag="os")
                            nc.vector.tensor_copy(osb[:], opx[:])
                            nc.sync.dma_start(
                                out[qsl, OH * nh + o0:OH * nh + o0 + ow],
                                osb[:])
    nc.compile()
    return nc


def host_inputs(cfg, x, k_cache, v_cache, Wq, Wo, core):
    import ml_dtypes
    bf16 = ml_dtypes.bfloat16
    c = _derived(cfg)
    CH, KB, KV, W, DH, DT, NP = (c["CH"], c["KB"], c["KV"], c["W"], c["DH"],
                                 c["DT"], c["NP"])
    b, ch = core // 4, core % 4
    Tc = k_cache.shape[2]
    f32 = np.float32

    xchunk = x[b, CH * ch:CH * (ch + 1), :].T.astype(f32)
    xT = np.ascontiguousarray(
        xchunk.reshape(DT, 128, CH).transpose(1, 0, 2)).astype(bf16)
    wq_s = (Wq.T.astype(f32) * f32(1.0 / np.sqrt(DH)))
    wqT = np.ascontiguousarray(
        wq_s.reshape(DT, 128, NP, 128).transpose(1, 2, 0, 3)).astype(bf16)
    woT = np.ascontiguousarray(
        Wo.T.astype(f32).reshape(NP, 128, c["D"]).transpose(1, 0, 2)
    ).astype(bf16)
    kw = k_cache[b, :, Tc - W:, :].astype(f32)
    kT64 = kw.reshape(KV, KB, 128, DH).transpose(3, 0, 1, 2)
    kaugm = np.zeros((64 + KB, KV, KB, 128), f32)
    kaugm[0:64] = kT64
    for kb in range(KB):
        kaugm[64 + kb, :, kb, :] = 1.0
    vw = v_cache[b, :, Tc - W:, :].astype(f32).reshape(KV, KB, 128, DH)
    vp = vw.transpose(2, 0, 1, 3)
    vaugm = np.ones((128, KV, KB, 65), f32)
    vaugm[:, :, :, :DH] = vp
    pos = (CH * ch + np.arange(CH)).astype(f32)
    inv = 1.0 / (cfg["BASE"] ** (np.arange(0, DH, 2, dtype=f32) / DH))
    r = np.arange(128)
    u = (r % 64) // 2
    ang = pos[None, :] * inv[u][:, None]
    cosT = np.cos(ang).astype(bf16)
    sinT = (np.sin(ang) * np.where(r % 2 == 0, -1.0, 1.0)[:, None]
            ).astype(bf16)
    if ch == 0:
        trim = (np.arange(128)[:, None] <= np.arange(128)[None, :]
                ).astype(f32)
        browm = np.zeros((KB, CH), f32)
        for kb in range(KB):
            browm[kb, :128 * kb] = BIGNEG
    else:
        trim = np.ones((128, 128), f32)
        browm = np.zeros((KB, CH), f32)
    sel2 = np.zeros((128, 256), f32)
    sel2[0, 0:128] = 1.0
    sel2[0, 192:256] = 1.0
    return {"xT": xT, "wqT": wqT, "woT": woT,
            "kaug": kaugm.astype(bf16), "vaug": vaugm.astype(bf16),
            "cosT": cosT, "sinT": sinT,
            "tri": trim.astype(bf16), "brow": browm.astype(bf16),
            "sel2": sel2.astype(bf16)}


_NC_CACHE = {}


def run(cfg, x, k_cache, v_cache, Wq, Wo, trace=False):
    key = tuple(sorted((k, v) for k, v in cfg.items()))
    if key not in _NC_CACHE:
        _NC_CACHE[key] = build(cfg)
    nc = _NC_CACHE[key]
    in_maps = [host_inputs(cfg, x, k_cache, v_cache, Wq, Wo, c)
               for c in range(8)]
    res = None
    for attempt in range(3):
        try:
            res = run_bass_kernel_spmd(nc, in_maps, core_ids=list(range(8)),
                                       trace=trace)
            break
        except Exception:
            if attempt == 2:
                raise
            time.sleep(2.0)
    outs = [res.results[c]["out"] for c in range(8)]
    full = np.stack([np.concatenate(outs[0:4], axis=0),
                     np.concatenate(outs[4:8], axis=0)])
    return full, res


def kernel(x, k_cache, v_cache, Wq, Wo):
    full, _ = run(FULL, np.asarray(x), np.asarray(k_cache),
                  np.asarray(v_cache), np.asarray(Wq), np.asarray(Wo))
    return full.astype(np.float32)
